# revision 38
# baseline (speedup 1.0000x reference)
"""Trainium2 Bass kernel: multi-head self-attention block (B=16, N=1024, C=768, H=12).

Data-parallel over batch: 8 NeuronCores x 2 batches each, no collectives.

Dataflow (per core, all-transposed activations; no on-chip transposes):
  host: xT = x_shard^T                                  [C, T]
  qkT  = W_qkv[:, :2C]^T-tiles @ xT                     [2C, T]   (q^T | k^T)
  v'   = xT-tiles^T @ W_qkv[:, 2C:]  (+ ones col/head)  [T, H*(HD+1)]
  S^T  = k^T-slices^T @ q^T   (per head, K=64)          [Nk, Nq]
  E    = exp(SCALE * S^T)     (ScalarE, PSUM->SBUF)
  U'   = v'^T @ E  (accum over k; row HD = softmax Z)   [HD+1, Nq]
  aoT  = U'[:HD] * (1/Z broadcast)                      [C, T]
  y    = aoT-tiles^T @ W_proj + b                       [T, C]

Schedule notes (from HW traces): the PE pays ~110ns whenever consecutive
matmuls change geometry, and the attention inner loop is Scalar(exp)-bound.
So attention runs as 24 q-half windows (hp, b, qn) whose U accumulators
need only 2 PSUM banks, kt steps are batched in PAIRS ([4xS][filler][4xU]
keeps same-shape runs at 4), U trails S by two pairs so exp latency is fully
hidden, and independent projection-group matmuls (qk of the next head pair,
v' of batch 1, output projection) are interleaved one group per pair to keep
the PE busy through the exp-bound stretches.
PSUM: 4 rotating S banks + 2 U banks + 2 filler banks.
"""

import sys

for _p in ("/opt/trn_rl_repo", "/opt/pypackages"):
    if _p not in sys.path:
        sys.path.append(_p)

import numpy as np

B, N, C, H = 16, 1024, 768, 12
HD = C // H            # 64
SCALE = HD ** -0.5
NCORES = 8
BL = B // NCORES       # 2 batches per core
T = BL * N             # 2048 tokens per core

COMPUTE = "bf16"       # "bf16" | "f32" | "f32r"


def build_attention_nc(compute=COMPUTE, bl=BL, n=N, c=C, h=H):
    import concourse.bass as bass
    import concourse.tile as tile
    from concourse import bacc, mybir
    from contextlib import ExitStack
    from collections import deque

    hd = c // h
    t = bl * n
    scale = hd ** -0.5
    assert c % 128 == 0 and n % 512 == 0 and h % 2 == 0 and hd == 64
    CCH = c // 128      # contraction chunks over channels (6)
    NHP = h // 2        # head pairs (6)
    NQ = n // 512       # 512-wide q tiles per sequence (2)
    NKT = n // 128      # 128-wide k tiles per sequence (8)
    NTT = n // 128      # 128-wide token tiles per sequence (8)
    NPAIR = NKT // 2    # kt pairs per half-window (4)
    VW = hd + 1         # v' width per head (ones col at hd)
    PH = c // 2         # proj/v free-dim half (384), <= 1 PSUM bank
    NXH = n // 512      # x halves per batch (2)
    assert PH <= 512

    FP32 = mybir.dt.float32
    SD = mybir.dt.bfloat16 if compute == "bf16" else FP32  # storage dtype

    def mm(ap):
        # matmul-operand view; f32r = fast single-pass fp32 path on TRN2 PE
        return ap.bitcast(mybir.dt.float32r) if compute == "f32r" else ap

    nc = bacc.Bacc("TRN2", target_bir_lowering=False, debug=False,
                   num_devices=NCORES)

    # inputs arrive pre-cast to the storage dtype (host-side cast)
    xT_d = nc.dram_tensor("xT", [c, t], SD, kind="ExternalInput").ap()
    wqkv_d = nc.dram_tensor("w_qkv", [c, 3 * c], SD, kind="ExternalInput").ap()
    wproj_d = nc.dram_tensor("w_proj", [c, c], SD, kind="ExternalInput").ap()
    bias_d = nc.dram_tensor("bias", [128, c], FP32, kind="ExternalInput").ap()
    out_d = nc.dram_tensor("out", [t, c], FP32, kind="ExternalOutput").ap()

    Exp = mybir.ActivationFunctionType.Exp

    with tile.TileContext(nc) as tc, ExitStack() as ctx:
        consts = ctx.enter_context(tc.tile_pool(name="consts", bufs=1))
        xp = ctx.enter_context(tc.tile_pool(name="xp", bufs=2))
        qkp = ctx.enter_context(tc.tile_pool(name="qkp", bufs=2))
        vp = ctx.enter_context(tc.tile_pool(name="vp", bufs=2))
        ep = ctx.enter_context(tc.tile_pool(name="ep", bufs=2))
        aop = ctx.enter_context(tc.tile_pool(name="aop", bufs=2))
        smp = ctx.enter_context(tc.tile_pool(name="smp", bufs=2))
        yp = ctx.enter_context(tc.tile_pool(name="yp", bufs=4))
        # PSUM: one 4-bank S pair-tile + 2 U accumulator banks + 2 filler banks
        ps_s = ctx.enter_context(tc.tile_pool(name="ps_s", bufs=1, space="PSUM"))
        ps_u = ctx.enter_context(tc.tile_pool(name="ps_u", bufs=2, space="PSUM"))
        ps_f = ctx.enter_context(tc.tile_pool(name="ps_f", bufs=2, space="PSUM"))

        # --- DMA staging, ordered by first use: wv + x(b0) first (v-phase),
        # then wqk (qk hp0), then x(b1) (v(b1) filler), then wproj/bias ---
        wqk_sb = []
        wv_sb = []
        wproj_sb = []
        xT_all = [[[None] * NXH for _ in range(CCH)] for _ in range(bl)]
        for cc in range(CCH):
            xt = xp.tile([128, 512], SD, tag=f"x{cc}_0", name=f"x_b0c{cc}h0")
            nc.sync.dma_start(out=xt, in_=xT_d[cc * 128:(cc + 1) * 128, 0:512])
            xT_all[0][cc][0] = xt
            wv = consts.tile([128, c], SD, tag=f"wv{cc}")
            nc.sync.dma_start(out=wv, in_=wqkv_d[cc * 128:(cc + 1) * 128,
                                                 2 * c:3 * c])
            wv_sb.append(wv)
        for cc in range(CCH):
            for xh in range(1, NXH):
                xt = xp.tile([128, 512], SD, tag=f"x{cc}_{xh}",
                             name=f"x_b0c{cc}h{xh}")
                nc.sync.dma_start(
                    out=xt, in_=xT_d[cc * 128:(cc + 1) * 128,
                                     xh * 512:(xh + 1) * 512])
                xT_all[0][cc][xh] = xt
            w1 = consts.tile([128, 2 * c], SD, tag=f"wqkv{cc}")
            nc.sync.dma_start(out=w1, in_=wqkv_d[cc * 128:(cc + 1) * 128,
                                                 0:2 * c])
            wqk_sb.append(w1)
        for b in range(1, bl):
            for cc in range(CCH):
                for xh in range(NXH):
                    xt = xp.tile([128, 512], SD, tag=f"x{cc}_{xh}",
                                 name=f"x_b{b}c{cc}h{xh}")
                    nc.sync.dma_start(
                        out=xt, in_=xT_d[cc * 128:(cc + 1) * 128,
                                         b * n + xh * 512:b * n + (xh + 1) * 512])
                    xT_all[b][cc][xh] = xt
        for cc in range(CCH):
            w2 = consts.tile([128, c], SD, tag=f"wproj{cc}")
            nc.sync.dma_start(out=w2, in_=wproj_d[cc * 128:(cc + 1) * 128, :])
            wproj_sb.append(w2)
        bias_sb = consts.tile([128, c], FP32, tag="bias")
        nc.sync.dma_start(out=bias_sb, in_=bias_d)

        # ---------- group emitters (each: one psum group + its evac) ----------
        # During upfront, groups rotate over the 2 f banks + 4 s banks for
        # deep pipelining; during the windows, fillers stick to the f banks.
        upfront = [True]
        ps_cycle = [0]

        def next_ps():
            if upfront[0]:
                # rotate over the 2 f banks + the (not yet live) 2 u banks
                ps_cycle[0] = (ps_cycle[0] + 1) % 4
                return (ps_f, "f") if ps_cycle[0] < 2 else (ps_u, "u")
            return (ps_f, "f")

        v_all = [[None] * NTT for _ in range(bl)]

        def v_tile_init(b, tt):
            vt = vp.tile([128, h * VW], SD, tag=f"v{tt}", name=f"v_b{b}t{tt}")
            ones_view = vt[:, :].rearrange("p (hh w) -> p hh w", hh=h)[:, :, hd:hd + 1]
            nc.gpsimd.memset(ones_view, 1.0)
            v_all[b][tt] = vt

        def v_group(b, tt, half):
            vt = v_all[b][tt]
            pool_, tg = next_ps()
            ps = pool_.tile([128, 512], FP32, tag=tg, name=f"vps_b{b}t{tt}f{half}")
            xh, tl = tt // 4, tt % 4
            for cc in range(CCH):
                nc.tensor.matmul(
                    ps[:, 0:PH],
                    lhsT=mm(xT_all[b][cc][xh][:, tl * 128:(tl + 1) * 128]),
                    rhs=mm(wv_sb[cc][:, half * PH:(half + 1) * PH]),
                    start=(cc == 0), stop=(cc == CCH - 1))
            # strided copy into per-head 64-wide slices (skip ones col)
            nheads = PH // hd
            dst = vt[:, half * nheads * VW:(half + 1) * nheads * VW].rearrange(
                "p (hh w) -> p hh w", hh=nheads)[:, :, 0:hd]
            srcv = ps[:, 0:PH].rearrange("p (hh w) -> p hh w", hh=nheads)
            with tc.high_priority(offset=300):
                nc.vector.tensor_copy(dst, srcv)

        qt_tiles = [None] * NHP
        kt_tiles = [None] * NHP

        def qk_tiles_init(hp):
            qt_tiles[hp] = qkp.tile([128, t], SD, tag="qt", name=f"qt{hp}")
            kt_tiles[hp] = qkp.tile([128, t], SD, tag="kt", name=f"kt{hp}")

        def qk_group(hp, dsti, qn):
            dst = (qt_tiles[hp], kt_tiles[hp])[dsti]
            fbase = dsti * c + hp * 128
            b_of = qn // (n // 512)
            qq = qn % (n // 512)
            pool_, tg = next_ps()
            ps = pool_.tile([128, 512], FP32, tag=tg, name=f"qkps{hp}_{dsti}_{qn}")
            for cc in range(CCH):
                nc.tensor.matmul(
                    ps,
                    lhsT=mm(wqk_sb[cc][:, fbase:fbase + 128]),
                    rhs=mm(xT_all[b_of][cc][qq]),
                    start=(cc == 0), stop=(cc == CCH - 1))
            with tc.high_priority(offset=300):
                nc.vector.tensor_copy(dst[:, qn * 512:(qn + 1) * 512], ps)

        aoT_all = [[None] * NHP for _ in range(bl)]

        def proj_group(b, tt, half):
            pool_, tg = next_ps()
            ps = pool_.tile([128, 512], FP32, tag=tg, name=f"yps_b{b}t{tt}f{half}")
            for cc in range(CCH):
                nc.tensor.matmul(
                    ps[:, 0:PH],
                    lhsT=mm(aoT_all[b][cc][:, tt * 128:(tt + 1) * 128]),
                    rhs=mm(wproj_sb[cc][:, half * PH:(half + 1) * PH]),
                    start=(cc == 0), stop=(cc == CCH - 1))
            yt = yp.tile([128, PH], FP32, tag="y", name=f"y_b{b}t{tt}f{half}")
            with tc.high_priority(offset=300):
                nc.vector.tensor_add(yt, ps[:, 0:PH],
                                     bias_sb[:, half * PH:(half + 1) * PH])
            nc.sync.dma_start(
                out=out_d[b * n + tt * 128:b * n + (tt + 1) * 128,
                          half * PH:(half + 1) * PH],
                in_=yt)

        # ---------- upfront: v'(b0) + qk(hp0) ----------
        for tt in range(NTT):
            v_tile_init(0, tt)
            for half in range(2):
                v_group(0, tt, half)
        qk_tiles_init(0)
        for dsti in range(2):
            for qn in range(t // 512):
                qk_group(0, dsti, qn)
        upfront[0] = False

        # ---------- filler queue ----------
        # entries are (deadline_gp, closure): the group must be emitted before
        # the global kt-pair slot with that index (force-drained there)
        fillers = deque()
        for tt in range(NTT):
            v_tile_init(1, tt)
        for tt in range(NTT):
            for half in range(2):
                fillers.append((2 * NPAIR,
                                lambda tt=tt, half=half: v_group(1, tt, half)))

        def push_qk(hp):
            qk_tiles_init(hp)
            for dsti in range(2):
                for qn in range(t // 512):
                    fillers.append((4 * hp * NPAIR,
                                    lambda hp=hp, dsti=dsti, qn=qn:
                                    qk_group(hp, dsti, qn)))

        def push_proj(b):
            for tt in range(NTT):
                for half in range(2):
                    fillers.append((1 << 30,
                                    lambda b=b, tt=tt, half=half:
                                    proj_group(b, tt, half)))

        # ---------- attention: 24 q-half windows, kt pairs, skew 2 ----------
        # Each pair's 4 S matmuls write one 4-bank PSUM tile drained by ONE
        # exp into one [128, 2048] E tile: the single producer/consumer makes
        # all 4 S (and later all 4 U) ready simultaneously, so the Tile
        # scheduler keeps them adjacent (head-alternating S co-execute on
        # disjoint PE row-tiles) instead of shredding the batches.
        ep_ctr = [0]

        def emit_S_pair(hp, b, qn, pair):
            # one 4-bank pair-tile drained by ONE exp: the single
            # producer/consumer makes all 4 S (and later all 4 U) ready
            # simultaneously, so the Tile scheduler keeps them adjacent
            # (head-alternating S co-execute on disjoint PE row-tiles)
            qb = qt_tiles[hp][:, b * n:(b + 1) * n]
            kb = kt_tiles[hp][:, b * n:(b + 1) * n]
            spair = ps_s.tile([128, 4 * 512], FP32, tag="s",
                              name=f"s_b{b}hp{hp}q{qn}p{pair}")
            g = 0
            for kt in (2 * pair, 2 * pair + 1):
                for head in range(2):
                    p0 = head * 64
                    nc.tensor.matmul(
                        spair[:, g * 512:(g + 1) * 512],
                        lhsT=mm(kb[p0:p0 + 64, kt * 128:(kt + 1) * 128]),
                        rhs=mm(qb[p0:p0 + 64, qn * 512:(qn + 1) * 512]),
                        start=True, stop=True)
                    g += 1
            et = ep.tile([128, 4 * 512], SD, tag=f"e{ep_ctr[0] % 3}", bufs=1,
                         name=f"e_b{b}hp{hp}q{qn}p{pair}")
            ep_ctr[0] += 1
            nc.scalar.activation(et, spair, Exp, scale=scale)
            return et

        def emit_U_pair(st):
            hp, b, qn, pair, et, u_t = st
            if not u_t:
                # lazy alloc: guarantees the bank-rotation reuse edge comes
                # after the previous half-window's final U writes and evacs
                u_t.extend(ps_u.tile([VW, 512], FP32, tag="u",
                                     name=f"u_b{b}hp{hp}q{qn}h{head}")
                           for head in range(2))
            g = 0
            for kt in (2 * pair, 2 * pair + 1):
                for head in range(2):
                    hh = 2 * hp + head
                    nc.tensor.matmul(
                        u_t[head],
                        lhsT=mm(v_all[b][kt][:, hh * VW:hh * VW + VW]),
                        rhs=mm(et[:, g * 512:(g + 1) * 512]),
                        start=(kt == 0), stop=(kt == NKT - 1))
                    g += 1

        def normalize(st):
            hp, b, qn, pair, ets, u_t = st
            if aoT_all[b][hp] is None:
                aoT_all[b][hp] = aop.tile([128, n], SD, tag=f"ao{hp}",
                                          name=f"ao_b{b}hp{hp}")
            ao = aoT_all[b][hp]
            cols = slice(qn * 512, (qn + 1) * 512)
            for head in (1, 0):
                usb = smp.tile([VW, 512], FP32, tag=f"usb{head}",
                               name=f"usb_b{b}hp{hp}q{qn}h{head}")
                # these copies gate U-accumulator bank release for the
                # next half-window: jump the DVE queue
                with tc.high_priority(offset=300):
                    nc.vector.tensor_copy(usb, u_t[head])
                # Z row -> partition 0 (DMA), broadcast to 64 partitions
                # (gpsimd), then reciprocal on the full-width tile (the
                # custom DVE op mis-executes on 1-partition slices at
                # base partition != 0).
                z1 = smp.tile([1, 512], FP32, tag=f"z1{head}", bufs=1,
                              name=f"z1_b{b}hp{hp}q{qn}h{head}")
                nc.gpsimd.dma_start(out=z1, in_=usb[hd:hd + 1, :])
                rb = smp.tile([64, 512], FP32, tag=f"rb{head}", bufs=1,
                              name=f"rb_b{b}hp{hp}q{qn}h{head}")
                nc.gpsimd.partition_broadcast(rb, z1)
                nc.vector.reciprocal_approx_fast(rb, rb)
                if head == 0:
                    nc.vector.tensor_mul(ao[0:64, cols], usb[0:hd, :], rb)
                else:
                    sc = smp.tile([64, 512], SD, tag="sc",
                                  name=f"sc_b{b}hp{hp}q{qn}")
                    nc.vector.tensor_mul(sc, usb[0:hd, :], rb)
                    nc.gpsimd.dma_start(out=ao[64:128, cols], in_=sc)

        halfwins = [(hp, b, qn) for hp in range(NHP) for b in range(bl)
                    for qn in range(NQ)]
        pend = deque()      # up to 2 pending (hp,b,qn,pair,ets,u_t)
        gp = 0      # global kt-pair slot counter
        for wi, (hp, b, qn) in enumerate(halfwins):
            if b == 0 and qn == 0 and hp + 1 < NHP:
                push_qk(hp + 1)
            u_t = []    # lazily allocated at first U emission
            for pair in range(NPAIR):
                # filler and U go FIRST: the PE queue is in-order, so the
                # S batch (which may wait on the previous pair's exp freeing
                # the single-buffered pair-tile) must sit BEHIND the work
                # that can cover that wait
                npop = 2 if wi < 2 else 1
                for _ in range(npop):
                    if fillers:
                        fillers.popleft()[1]()
                # safety: emit anything the next slots' U/S depend on
                while fillers and fillers[0][0] <= gp + 1:
                    fillers.popleft()[1]()
                gp += 1
                if len(pend) == 2:
                    st = pend.popleft()
                    emit_U_pair(st)
                    if st[3] == NPAIR - 1:
                        normalize(st)
                        if st[0] == NHP - 1 and st[2] == NQ - 1:
                            push_proj(st[1])
                ets = emit_S_pair(hp, b, qn, pair)
                pend.append((hp, b, qn, pair, ets, u_t))
        while pend:
            st = pend.popleft()
            emit_U_pair(st)
            if st[3] == NPAIR - 1:
                normalize(st)
                if st[0] == NHP - 1 and st[2] == NQ - 1:
                    push_proj(st[1])
        while fillers:
            fillers.popleft()[1]()

    nc.compile()
    return nc


_NC_CACHE = {}


def _get_nc(compute=COMPUTE):
    if compute not in _NC_CACHE:
        _NC_CACHE[compute] = build_attention_nc(compute)
    return _NC_CACHE[compute]


def make_in_maps(x, W_qkv, W_proj, b_proj, compute=None):
    compute = compute or COMPUTE
    if compute == "bf16":
        import ml_dtypes
        sd = ml_dtypes.bfloat16
    else:
        sd = np.float32
    x = np.asarray(x, dtype=np.float32)
    W_qkv = np.ascontiguousarray(np.asarray(W_qkv, dtype=np.float32)).astype(sd)
    W_proj = np.ascontiguousarray(np.asarray(W_proj, dtype=np.float32)).astype(sd)
    bias = np.ascontiguousarray(
        np.broadcast_to(np.asarray(b_proj, dtype=np.float32), (128, C)))
    in_maps = []
    for i in range(NCORES):
        shard = x[i * BL:(i + 1) * BL]                      # [BL, N, C]
        xT = np.ascontiguousarray(shard.transpose(2, 0, 1).reshape(C, T)).astype(sd)
        in_maps.append({"xT": xT, "w_qkv": W_qkv, "w_proj": W_proj,
                        "bias": bias})
    return in_maps


def kernel(x, W_qkv, W_proj, b_proj):
    from concourse.bass_utils import run_bass_kernel_spmd

    nc = _get_nc()
    in_maps = make_in_maps(x, W_qkv, W_proj, b_proj)
    res = run_bass_kernel_spmd(nc, in_maps, core_ids=list(range(NCORES)))
    outs = [res.results[i]["out"].reshape(BL, N, C) for i in range(NCORES)]
    return np.concatenate(outs, axis=0).astype(np.float32)


if __name__ == "__main__":
    nc = build_attention_nc()
    print("built ok")


# revision 40
# speedup vs baseline: 1.1887x; 1.1887x over previous
"""Trainium2 Bass kernel: multi-head self-attention block (B=16, N=1024, C=768, H=12).

Data-parallel over batch: 8 NeuronCores x 2 batches each, no collectives.

Dataflow (per core, all-transposed activations; no on-chip transposes):
  host: xT = x_shard^T                                  [C, T]
  qkT  = W_qkv[:, :2C]^T-tiles @ xT                     [2C, T]   (q^T | k^T)
  v'   = xT-tiles^T @ W_qkv[:, 2C:]  (+ ones col/head)  [T, H*(HD+1)]
  S^T  = k^T-slices^T @ q^T   (per head, K=64)          [Nk, Nq]
  E    = exp(SCALE * S^T)     (ScalarE, PSUM->SBUF)
  U'   = v'^T @ E  (accum over k; row HD = softmax Z)   [HD+1, Nq]
  aoT  = U'[:HD] * (1/Z broadcast)                      [C, T]
  y    = aoT-tiles^T @ W_proj + b                       [T, C]

Schedule notes (from HW traces): the PE pays ~110ns whenever consecutive
matmuls change geometry, and the attention inner loop is Scalar(exp)-bound.
So attention runs as 24 q-half windows (hp, b, qn) whose U accumulators
need only 2 PSUM banks, kt steps are batched in PAIRS ([4xS][filler][4xU]
keeps same-shape runs at 4), U trails S by two pairs so exp latency is fully
hidden, and independent projection-group matmuls (qk of the next head pair,
v' of batch 1, output projection) are interleaved one group per pair to keep
the PE busy through the exp-bound stretches.
PSUM: 4 rotating S banks + 2 U banks + 2 filler banks.
"""

import sys

for _p in ("/opt/trn_rl_repo", "/opt/pypackages"):
    if _p not in sys.path:
        sys.path.append(_p)

import numpy as np

B, N, C, H = 16, 1024, 768, 12
HD = C // H            # 64
SCALE = HD ** -0.5
NCORES = 8
BL = B // NCORES       # 2 batches per core
T = BL * N             # 2048 tokens per core

COMPUTE = "bf16"       # "bf16" | "f32" | "f32r"


def build_attention_nc(compute=COMPUTE, bl=BL, n=N, c=C, h=H):
    import concourse.bass as bass
    import concourse.tile as tile
    from concourse import bacc, mybir
    from contextlib import ExitStack
    from collections import deque

    hd = c // h
    t = bl * n
    scale = hd ** -0.5
    assert c % 128 == 0 and n % 512 == 0 and h % 2 == 0 and hd == 64
    CCH = c // 128      # contraction chunks over channels (6)
    NHP = h // 2        # head pairs (6)
    NQ = n // 512       # 512-wide q tiles per sequence (2)
    NKT = n // 128      # 128-wide k tiles per sequence (8)
    NTT = n // 128      # 128-wide token tiles per sequence (8)
    NPAIR = NKT // 2    # kt pairs per half-window (4)
    VW = hd + 1         # v' width per head (ones col at hd)
    PH = c // 2         # proj/v free-dim half (384), <= 1 PSUM bank
    NXH = n // 512      # x halves per batch (2)
    assert PH <= 512

    FP32 = mybir.dt.float32
    SD = mybir.dt.bfloat16 if compute == "bf16" else FP32  # storage dtype

    def mm(ap):
        # matmul-operand view; f32r = fast single-pass fp32 path on TRN2 PE
        return ap.bitcast(mybir.dt.float32r) if compute == "f32r" else ap

    nc = bacc.Bacc("TRN2", target_bir_lowering=False, debug=False,
                   num_devices=NCORES)

    # inputs arrive pre-cast to the storage dtype (host-side cast)
    xT_d = nc.dram_tensor("xT", [c, t], SD, kind="ExternalInput").ap()
    wqkv_d = nc.dram_tensor("w_qkv", [c, 3 * c], SD, kind="ExternalInput").ap()
    wproj_d = nc.dram_tensor("w_proj", [c, c], SD, kind="ExternalInput").ap()
    bias_d = nc.dram_tensor("bias", [128, c], FP32, kind="ExternalInput").ap()
    out_d = nc.dram_tensor("out", [t, c], FP32, kind="ExternalOutput").ap()

    Exp = mybir.ActivationFunctionType.Exp

    with tile.TileContext(nc) as tc, ExitStack() as ctx:
        consts = ctx.enter_context(tc.tile_pool(name="consts", bufs=1))
        xp = ctx.enter_context(tc.tile_pool(name="xp", bufs=2))
        qkp = ctx.enter_context(tc.tile_pool(name="qkp", bufs=2))
        vp = ctx.enter_context(tc.tile_pool(name="vp", bufs=2))
        ep = ctx.enter_context(tc.tile_pool(name="ep", bufs=2))
        aop = ctx.enter_context(tc.tile_pool(name="aop", bufs=2))
        smp = ctx.enter_context(tc.tile_pool(name="smp", bufs=2))
        yp = ctx.enter_context(tc.tile_pool(name="yp", bufs=4))
        # PSUM: one 4-bank S pair-tile + 2 U accumulator banks + 2 filler banks
        ps_s = ctx.enter_context(tc.tile_pool(name="ps_s", bufs=1, space="PSUM"))
        ps_u = ctx.enter_context(tc.tile_pool(name="ps_u", bufs=2, space="PSUM"))
        ps_f = ctx.enter_context(tc.tile_pool(name="ps_f", bufs=2, space="PSUM"))

        # --- DMA staging, ordered by first use: wv + x(b0) first (v-phase),
        # then wqk (qk hp0), then x(b1) (v(b1) filler), then wproj/bias ---
        wqk_sb = []
        wv_sb = []
        wproj_sb = []
        xT_all = [[[None] * NXH for _ in range(CCH)] for _ in range(bl)]
        for cc in range(CCH):
            xt = xp.tile([128, 512], SD, tag=f"x{cc}_0", name=f"x_b0c{cc}h0")
            nc.sync.dma_start(out=xt, in_=xT_d[cc * 128:(cc + 1) * 128, 0:512])
            xT_all[0][cc][0] = xt
            wv = consts.tile([128, c], SD, tag=f"wv{cc}")
            nc.sync.dma_start(out=wv, in_=wqkv_d[cc * 128:(cc + 1) * 128,
                                                 2 * c:3 * c])
            wv_sb.append(wv)
        for cc in range(CCH):
            for xh in range(1, NXH):
                xt = xp.tile([128, 512], SD, tag=f"x{cc}_{xh}",
                             name=f"x_b0c{cc}h{xh}")
                nc.sync.dma_start(
                    out=xt, in_=xT_d[cc * 128:(cc + 1) * 128,
                                     xh * 512:(xh + 1) * 512])
                xT_all[0][cc][xh] = xt
            w1 = consts.tile([128, 2 * c], SD, tag=f"wqkv{cc}")
            nc.sync.dma_start(out=w1, in_=wqkv_d[cc * 128:(cc + 1) * 128,
                                                 0:2 * c])
            wqk_sb.append(w1)
        for b in range(1, bl):
            for cc in range(CCH):
                for xh in range(NXH):
                    xt = xp.tile([128, 512], SD, tag=f"x{cc}_{xh}",
                                 name=f"x_b{b}c{cc}h{xh}")
                    nc.sync.dma_start(
                        out=xt, in_=xT_d[cc * 128:(cc + 1) * 128,
                                         b * n + xh * 512:b * n + (xh + 1) * 512])
                    xT_all[b][cc][xh] = xt
        for cc in range(CCH):
            w2 = consts.tile([128, c], SD, tag=f"wproj{cc}")
            nc.sync.dma_start(out=w2, in_=wproj_d[cc * 128:(cc + 1) * 128, :])
            wproj_sb.append(w2)
        bias_sb = consts.tile([128, c], FP32, tag="bias")
        nc.sync.dma_start(out=bias_sb, in_=bias_d)

        # ---------- group emitters (each: one psum group + its evac) ----------
        # During upfront, groups rotate over the 2 f banks + 4 s banks for
        # deep pipelining; during the windows, fillers stick to the f banks.
        upfront = [True]
        ps_cycle = [0]

        def next_ps():
            if upfront[0]:
                # rotate over the 2 f banks + the (not yet live) 2 u banks
                ps_cycle[0] = (ps_cycle[0] + 1) % 4
                return (ps_f, "f") if ps_cycle[0] < 2 else (ps_u, "u")
            return (ps_f, "f")

        v_all = [[None] * NTT for _ in range(bl)]

        def v_tile_init(b, tt):
            vt = vp.tile([128, h * VW], SD, tag=f"v{tt}", name=f"v_b{b}t{tt}")
            ones_view = vt[:, :].rearrange("p (hh w) -> p hh w", hh=h)[:, :, hd:hd + 1]
            nc.gpsimd.memset(ones_view, 1.0)
            v_all[b][tt] = vt

        def v_group(b, tt, half):
            vt = v_all[b][tt]
            pool_, tg = next_ps()
            ps = pool_.tile([128, 512], FP32, tag=tg, name=f"vps_b{b}t{tt}f{half}")
            xh, tl = tt // 4, tt % 4
            for cc in range(CCH):
                nc.tensor.matmul(
                    ps[:, 0:PH],
                    lhsT=mm(xT_all[b][cc][xh][:, tl * 128:(tl + 1) * 128]),
                    rhs=mm(wv_sb[cc][:, half * PH:(half + 1) * PH]),
                    start=(cc == 0), stop=(cc == CCH - 1))
            # strided copy into per-head 64-wide slices (skip ones col)
            nheads = PH // hd
            dst = vt[:, half * nheads * VW:(half + 1) * nheads * VW].rearrange(
                "p (hh w) -> p hh w", hh=nheads)[:, :, 0:hd]
            srcv = ps[:, 0:PH].rearrange("p (hh w) -> p hh w", hh=nheads)
            with tc.high_priority(offset=300):
                nc.vector.tensor_copy(dst, srcv)

        qt_tiles = [None] * NHP
        kt_tiles = [None] * NHP

        def qk_tiles_init(hp):
            qt_tiles[hp] = qkp.tile([128, t], SD, tag="qt", name=f"qt{hp}")
            kt_tiles[hp] = qkp.tile([128, t], SD, tag="kt", name=f"kt{hp}")

        def qk_group(hp, dsti, qn):
            dst = (qt_tiles[hp], kt_tiles[hp])[dsti]
            fbase = dsti * c + hp * 128
            b_of = qn // (n // 512)
            qq = qn % (n // 512)
            pool_, tg = next_ps()
            ps = pool_.tile([128, 512], FP32, tag=tg, name=f"qkps{hp}_{dsti}_{qn}")
            for cc in range(CCH):
                nc.tensor.matmul(
                    ps,
                    lhsT=mm(wqk_sb[cc][:, fbase:fbase + 128]),
                    rhs=mm(xT_all[b_of][cc][qq]),
                    start=(cc == 0), stop=(cc == CCH - 1))
            with tc.high_priority(offset=300):
                nc.vector.tensor_copy(dst[:, qn * 512:(qn + 1) * 512], ps)

        aoT_all = [[None] * NHP for _ in range(bl)]

        def proj_group(b, tt, half):
            pool_, tg = next_ps()
            ps = pool_.tile([128, 512], FP32, tag=tg, name=f"yps_b{b}t{tt}f{half}")
            for cc in range(CCH):
                nc.tensor.matmul(
                    ps[:, 0:PH],
                    lhsT=mm(aoT_all[b][cc][:, tt * 128:(tt + 1) * 128]),
                    rhs=mm(wproj_sb[cc][:, half * PH:(half + 1) * PH]),
                    start=(cc == 0), stop=(cc == CCH - 1))
            yt = yp.tile([128, PH], FP32, tag="y", name=f"y_b{b}t{tt}f{half}")
            with tc.high_priority(offset=300):
                nc.vector.tensor_add(yt, ps[:, 0:PH],
                                     bias_sb[:, half * PH:(half + 1) * PH])
            nc.sync.dma_start(
                out=out_d[b * n + tt * 128:b * n + (tt + 1) * 128,
                          half * PH:(half + 1) * PH],
                in_=yt)

        # ---------- upfront: v'(b0) + qk(hp0) ----------
        for tt in range(NTT):
            v_tile_init(0, tt)
            for half in range(2):
                v_group(0, tt, half)
        qk_tiles_init(0)
        for dsti in range(2):
            for qn in range(t // 512):
                qk_group(0, dsti, qn)
        upfront[0] = False

        # ---------- filler queue ----------
        # entries are (deadline_gp, closure): the group must be emitted before
        # the global kt-pair slot with that index (force-drained there)
        fillers = deque()
        for tt in range(NTT):
            v_tile_init(1, tt)
        for tt in range(NTT):
            for half in range(2):
                fillers.append((2 * NPAIR,
                                lambda tt=tt, half=half: v_group(1, tt, half)))

        def push_qk(hp):
            qk_tiles_init(hp)
            for dsti in range(2):
                for qn in range(t // 512):
                    fillers.append((4 * hp * NPAIR,
                                    lambda hp=hp, dsti=dsti, qn=qn:
                                    qk_group(hp, dsti, qn)))

        def push_proj(b):
            for tt in range(NTT):
                for half in range(2):
                    fillers.append((1 << 30,
                                    lambda b=b, tt=tt, half=half:
                                    proj_group(b, tt, half)))

        # ---------- attention: 24 q-half windows, kt pairs, skew 2 ----------
        # Each pair's 4 S matmuls write one 4-bank PSUM tile drained by ONE
        # exp into one [128, 2048] E tile: the single producer/consumer makes
        # all 4 S (and later all 4 U) ready simultaneously, so the Tile
        # scheduler keeps them adjacent (head-alternating S co-execute on
        # disjoint PE row-tiles) instead of shredding the batches.
        ep_ctr = [0]

        def emit_S_pair(hp, b, qn, pair):
            # one 4-bank pair-tile drained by ONE exp: the single
            # producer/consumer makes all 4 S (and later all 4 U) ready
            # simultaneously, so the Tile scheduler keeps them adjacent
            # (head-alternating S co-execute on disjoint PE row-tiles)
            qb = qt_tiles[hp][:, b * n:(b + 1) * n]
            kb = kt_tiles[hp][:, b * n:(b + 1) * n]
            spair = ps_s.tile([128, 4 * 512], FP32, tag="s",
                              name=f"s_b{b}hp{hp}q{qn}p{pair}")
            g = 0
            for kt in (2 * pair, 2 * pair + 1):
                for head in range(2):
                    p0 = head * 64
                    nc.tensor.matmul(
                        spair[:, g * 512:(g + 1) * 512],
                        lhsT=mm(kb[p0:p0 + 64, kt * 128:(kt + 1) * 128]),
                        rhs=mm(qb[p0:p0 + 64, qn * 512:(qn + 1) * 512]),
                        start=True, stop=True)
                    g += 1
            et = ep.tile([128, 4 * 512], SD, tag=f"e{ep_ctr[0] % 4}", bufs=1,
                         name=f"e_b{b}hp{hp}q{qn}p{pair}")
            ep_ctr[0] += 1
            nc.scalar.activation(et, spair, Exp, scale=scale)
            return et

        def emit_U_pair(st):
            hp, b, qn, pair, et, u_t = st
            if not u_t:
                # lazy alloc: guarantees the bank-rotation reuse edge comes
                # after the previous half-window's final U writes and evacs
                u_t.extend(ps_u.tile([VW, 512], FP32, tag="u",
                                     name=f"u_b{b}hp{hp}q{qn}h{head}")
                           for head in range(2))
            g = 0
            for kt in (2 * pair, 2 * pair + 1):
                for head in range(2):
                    hh = 2 * hp + head
                    nc.tensor.matmul(
                        u_t[head],
                        lhsT=mm(v_all[b][kt][:, hh * VW:hh * VW + VW]),
                        rhs=mm(et[:, g * 512:(g + 1) * 512]),
                        start=(kt == 0), stop=(kt == NKT - 1))
                    g += 1

        def normalize(st):
            hp, b, qn, pair, ets, u_t = st
            if aoT_all[b][hp] is None:
                aoT_all[b][hp] = aop.tile([128, n], SD, tag=f"ao{hp}",
                                          name=f"ao_b{b}hp{hp}")
            ao = aoT_all[b][hp]
            cols = slice(qn * 512, (qn + 1) * 512)
            for head in (1, 0):
                usb = smp.tile([VW, 512], FP32, tag=f"usb{head}",
                               name=f"usb_b{b}hp{hp}q{qn}h{head}")
                # these copies gate U-accumulator bank release for the
                # next half-window: jump the DVE queue
                with tc.high_priority(offset=300):
                    nc.vector.tensor_copy(usb, u_t[head])
                # Z row -> partition 0 (DMA), broadcast to 64 partitions
                # (gpsimd), then reciprocal on the full-width tile (the
                # custom DVE op mis-executes on 1-partition slices at
                # base partition != 0).
                z1 = smp.tile([1, 512], FP32, tag=f"z1{head}", bufs=1,
                              name=f"z1_b{b}hp{hp}q{qn}h{head}")
                nc.gpsimd.dma_start(out=z1, in_=usb[hd:hd + 1, :])
                rb = smp.tile([64, 512], FP32, tag=f"rb{head}", bufs=1,
                              name=f"rb_b{b}hp{hp}q{qn}h{head}")
                nc.gpsimd.partition_broadcast(rb, z1)
                nc.vector.reciprocal_approx_fast(rb, rb)
                if head == 0:
                    nc.vector.tensor_mul(ao[0:64, cols], usb[0:hd, :], rb)
                else:
                    sc = smp.tile([64, 512], SD, tag="sc",
                                  name=f"sc_b{b}hp{hp}q{qn}")
                    nc.vector.tensor_mul(sc, usb[0:hd, :], rb)
                    nc.gpsimd.dma_start(out=ao[64:128, cols], in_=sc)

        halfwins = [(hp, b, qn) for hp in range(NHP) for b in range(bl)
                    for qn in range(NQ)]
        pend = deque()      # up to 2 pending (hp,b,qn,pair,ets,u_t)
        gp = 0      # global kt-pair slot counter
        for wi, (hp, b, qn) in enumerate(halfwins):
            if b == 0 and qn == 0 and hp + 1 < NHP:
                push_qk(hp + 1)
            u_t = []    # lazily allocated at first U emission
            for pair in range(NPAIR):
                ets = emit_S_pair(hp, b, qn, pair)
                npop = 2 if wi < 2 else 1
                for _ in range(npop):
                    if fillers:
                        fillers.popleft()[1]()
                # safety: emit anything the next slots' U/S depend on
                while fillers and fillers[0][0] <= gp + 1:
                    fillers.popleft()[1]()
                gp += 1
                if len(pend) == 2:
                    st = pend.popleft()
                    emit_U_pair(st)
                    if st[3] == NPAIR - 1:
                        normalize(st)
                        if st[0] == NHP - 1 and st[2] == NQ - 1:
                            push_proj(st[1])
                pend.append((hp, b, qn, pair, ets, u_t))
        while pend:
            st = pend.popleft()
            emit_U_pair(st)
            if st[3] == NPAIR - 1:
                normalize(st)
                if st[0] == NHP - 1 and st[2] == NQ - 1:
                    push_proj(st[1])
        while fillers:
            fillers.popleft()[1]()

    nc.compile()
    return nc


_NC_CACHE = {}


def _get_nc(compute=COMPUTE):
    if compute not in _NC_CACHE:
        _NC_CACHE[compute] = build_attention_nc(compute)
    return _NC_CACHE[compute]


def make_in_maps(x, W_qkv, W_proj, b_proj, compute=None):
    compute = compute or COMPUTE
    if compute == "bf16":
        import ml_dtypes
        sd = ml_dtypes.bfloat16
    else:
        sd = np.float32
    x = np.asarray(x, dtype=np.float32)
    W_qkv = np.ascontiguousarray(np.asarray(W_qkv, dtype=np.float32)).astype(sd)
    W_proj = np.ascontiguousarray(np.asarray(W_proj, dtype=np.float32)).astype(sd)
    bias = np.ascontiguousarray(
        np.broadcast_to(np.asarray(b_proj, dtype=np.float32), (128, C)))
    in_maps = []
    for i in range(NCORES):
        shard = x[i * BL:(i + 1) * BL]                      # [BL, N, C]
        xT = np.ascontiguousarray(shard.transpose(2, 0, 1).reshape(C, T)).astype(sd)
        in_maps.append({"xT": xT, "w_qkv": W_qkv, "w_proj": W_proj,
                        "bias": bias})
    return in_maps


def kernel(x, W_qkv, W_proj, b_proj):
    from concourse.bass_utils import run_bass_kernel_spmd

    nc = _get_nc()
    in_maps = make_in_maps(x, W_qkv, W_proj, b_proj)
    res = run_bass_kernel_spmd(nc, in_maps, core_ids=list(range(NCORES)))
    outs = [res.results[i]["out"].reshape(BL, N, C) for i in range(NCORES)]
    return np.concatenate(outs, axis=0).astype(np.float32)


if __name__ == "__main__":
    nc = build_attention_nc()
    print("built ok")
